# revision 23
# baseline (speedup 1.0000x reference)
"""Bass/Tile attention kernel for TRN2 — per-core program builder (v2).

Sharding (SPMD, core c of 8): batch b = c//2, head-half hh = c%2.
Each core computes Q/K/V projections for its 8 heads only (512 of the
1024 model dims), full attention for those heads over all 2048 tokens,
and a PARTIAL output projection O_part = Z_local @ Wo[local_rows].
The host sums the two partials per batch and adds bo (no collective).

Per-core DRAM inputs:
  xt  : [D, S]   bf16  X[b]^T (full model dims, all tokens)
  wq/wk/wv : [D, DL]  bf16  column-slice for this head-half
  wo  : [DL, D]  bf16  row-slice for this head-half
  bq/bk/bv : [DL] fp32
Output:
  ot  : [D, S]  fp32  partial O^T for this batch (host: sum pair + ^T + bo)

On-chip layouts (P=128 partitions):
  xt_sb[p, c, t] = X^T[c*128+p, t]                  bf16
  kt[pr][p, t]   = K^T[pr*128+p, t]   (pr = local pair of heads)
  qt2[qb][p, pr, q] = Q^T[pr*128+p, qc*512+q]  (double-buffered per qc)
  vt[pr][p, tk, h*65+j] = V[tk*128+p, pr*128+h*64+j] for j<64,
                          1.0 for j==64 (ones col -> softmax denom)
  zt[pr][p, q]   = normalized Z^T

Attention inner loop per (qc, pr): 16 kc chunks; per kc two row-packed
score matmuls (contraction 64 per head, PE rows 0-63 / 64-127) into a
2-bank PSUM tile, one ACT exp -> bf16 probs, and two M=65 PV matmuls
accumulating za/zb (row 64 = denominator).  PV is emitted LAG=2 kc
behind scores so the PE stream never blocks on the scalar engine.
Q-proj for the next qc and O-proj for the previous qc are emitted as
filler chains inside the kc stream (separate 1-bank PSUM pool).
"""

import numpy as np
import ml_dtypes
from contextlib import ExitStack

import concourse.bass as bass
import concourse.tile as tile
from concourse import bacc, mybir, library_config

GPB = True   # normalize via gpsimd partition_broadcast (else DRAM bounce)

F32 = mybir.dt.float32
BF16 = mybir.dt.bfloat16
P = 128


def build_attention_nc(S=2048, D=1024, DL=512):
    NPAIR = DL // P           # 4 local head pairs
    DIN = D // P              # 8 input-dim chunks
    KC = S // P               # 16 key chunks
    QCN = S // 512            # 4 query chunks
    NTOK = S // P             # 16 token chunks (V proj)
    LAG = 2                   # PV lags scores by LAG kc steps

    nc = bacc.Bacc("TRN2", target_bir_lowering=False, debug=False)

    xt_d = nc.dram_tensor("xt", [D, S], BF16, kind="ExternalInput").ap()
    wq_d = nc.dram_tensor("wq", [D, DL], BF16, kind="ExternalInput").ap()
    wk_d = nc.dram_tensor("wk", [D, DL], BF16, kind="ExternalInput").ap()
    wv_d = nc.dram_tensor("wv", [D, DL], BF16, kind="ExternalInput").ap()
    wo_d = nc.dram_tensor("wo", [DL, D], BF16, kind="ExternalInput").ap()
    bq_d = nc.dram_tensor("bq", [DL], F32, kind="ExternalInput").ap()
    bk_d = nc.dram_tensor("bk", [DL], F32, kind="ExternalInput").ap()
    bv_d = nc.dram_tensor("bv", [DL], F32, kind="ExternalInput").ap()
    ot_d = nc.dram_tensor("ot", [D, S], F32, kind="ExternalOutput").ap()

    xt_r = xt_d.rearrange("(c p) t -> p c t", p=P)
    wq_r = wq_d.rearrange("(c p) n -> p c n", p=P)
    wk_r = wk_d.rearrange("(c p) n -> p c n", p=P)
    wv_r = wv_d.rearrange("(c p) n -> p c n", p=P)
    wo_r = wo_d.rearrange("(c p) n -> p c n", p=P)
    bq_r = bq_d.rearrange("(c p) -> p c", p=P)
    bk_r = bk_d.rearrange("(c p) -> p c", p=P)
    bv_r = bv_d.rearrange("(a d) -> a d", a=1)

    EXP = mybir.ActivationFunctionType.Exp

    with tile.TileContext(nc) as tc, ExitStack() as ctx:
        const = ctx.enter_context(tc.tile_pool(name="const", bufs=1))
        big = ctx.enter_context(tc.tile_pool(name="big", bufs=1))
        wpool = ctx.enter_context(tc.tile_pool(name="wpool", bufs=4))
        qpool = ctx.enter_context(tc.tile_pool(name="qpool", bufs=2))
        work = ctx.enter_context(tc.tile_pool(name="work", bufs=3))
        probs_pool = ctx.enter_context(tc.tile_pool(name="probs", bufs=5))
        # PSUM budget (8 banks): scores 2x2 + z 2 + proj-filler 2
        spsum = ctx.enter_context(tc.tile_pool(name="spsum", bufs=2, space="PSUM"))
        zpsum = ctx.enter_context(tc.tile_pool(name="zpsum", bufs=2, space="PSUM"))
        ppsum = ctx.enter_context(tc.tile_pool(name="ppsum", bufs=2, space="PSUM"))
        dramp = ctx.enter_context(tc.tile_pool(name="dramp", bufs=2, space="DRAM"))

        # ---- constants ----
        bq_t = const.tile([P, NPAIR], F32)
        nc.sync.dma_start(bq_t[:], bq_r[:, :])
        bk_t = const.tile([P, NPAIR], F32)
        nc.sync.dma_start(bk_t[:], bk_r[:, :])
        bvb = const.tile([P, DL], F32)
        nc.sync.dma_start(bvb[:], bv_r[0:1, :].to_broadcast((P, DL)))

        if GPB:
            nc.gpsimd.load_library(library_config.attn)

        # ---- batched input DMAs, front-loaded so the first K-proj chain
        # starts ~5us in: wk + xt token-piece 0 first (halved for queue
        # parallelism), then wq (first q-proj), wv, remaining xt, wo ----
        xt_t = big.tile([P, DIN, S], BF16, tag="xt")
        wk_t = wpool.tile([P, DIN, DL], BF16, tag="w", name="wk_t")
        wq_t = wpool.tile([P, DIN, DL], BF16, tag="w", name="wq_t")
        wv_t = wpool.tile([P, DIN, DL], BF16, tag="w", name="wv_t")
        wo_t = wpool.tile([P, NPAIR, D], BF16, tag="w", name="wo_t")
        for c in range(DIN):
            nc.sync.dma_start(wk_t[:, c, :], wk_r[:, c, :])
            nc.sync.dma_start(xt_t[:, c, 0:512], xt_r[:, c, 0:512])
        for c in range(DIN):
            nc.sync.dma_start(wq_t[:, c, :], wq_r[:, c, :])
        for c in range(DIN):
            nc.sync.dma_start(wv_t[:, c, :], wv_r[:, c, :])
        for t in range(1, QCN):
            tsl = slice(t * 512, (t + 1) * 512)
            for c in range(DIN):
                nc.sync.dma_start(xt_t[:, c, tsl], xt_r[:, c, tsl])
        nc.sync.dma_start(wo_t[:, 0:2, :], wo_r[:, 0:2, :])
        nc.sync.dma_start(wo_t[:, 2:NPAIR, :], wo_r[:, 2:NPAIR, :])

        kt = [big.tile([P, S], BF16, tag=f"kt{p}", name=f"kt{p}")
              for p in range(NPAIR)]
        vt = [big.tile([P, NTOK, 130], BF16, tag=f"vt{p}", name=f"vt{p}")
              for p in range(NPAIR)]
        zt = [big.tile([P, S], BF16, tag=f"zt{p}", name=f"zt{p}")
              for p in range(NPAIR)]

        # ones columns of vt (col 64 of each head's 65-wide block)
        for pr in range(NPAIR):
            ones_ap = vt[pr].rearrange("p t (h c) -> p t h c", c=65)[:, :, :, 64:65]
            nc.vector.memset(ones_ap, 1.0)

        # ---- projection chains, emitted just-in-time inside the
        # ACT-paced attention stream (1-bank ppsum pool each) ----
        def kproj_chain(pr, t):
            ps = ppsum.tile([P, 512], F32, tag="proj", name="psk")
            for c in range(DIN):
                nc.tensor.matmul(
                    ps[:],
                    wk_t[:, c, pr * P:(pr + 1) * P],
                    xt_t[:, c, t * 512:(t + 1) * 512],
                    start=(c == 0), stop=(c == DIN - 1),
                )
            nc.vector.tensor_scalar_add(
                kt[pr][:, t * 512:(t + 1) * 512], ps[:], bk_t[:, pr:pr + 1])

        def vproj_chain(t):
            ps = ppsum.tile([P, 512], F32, tag="proj", name="psv")
            for c in range(DIN):
                nc.tensor.matmul(
                    ps[:],
                    xt_t[:, c, t * P:(t + 1) * P],
                    wv_t[:, c, :],
                    start=(c == 0), stop=(c == DIN - 1),
                )
            for pr in range(NPAIR):
                dst = vt[pr].rearrange(
                    "p t (h c) -> p t h c", c=65)[:, t, :, 0:64]
                src = ps[:, pr * P:(pr + 1) * P].rearrange(
                    "p (h c) -> p h c", c=64)
                bsrc = bvb[:, pr * P:(pr + 1) * P].rearrange(
                    "p (h c) -> p h c", c=64)
                nc.vector.tensor_add(dst, src, bsrc)

        # ---- filler chain generators (run inside the kc stream) ----
        def oproj_chain(qc, ec, tail=False):
            if tail and ec % 2 == 0:
                # attention is over: alternate with the scores pool so the
                # DVE evictions double-buffer instead of serializing
                pst = spsum.tile([P, 2, 512], F32, tag="sc", name="psot")
                ps = pst[:, 0, :]
            else:
                ps = ppsum.tile([P, 512], F32, tag="proj", name="pso")[:]
            for dc in range(NPAIR):
                nc.tensor.matmul(
                    ps,
                    wo_t[:, dc, ec * P:(ec + 1) * P],
                    zt[dc][:, qc * 512:(qc + 1) * 512],
                    start=(dc == 0), stop=(dc == NPAIR - 1),
                )
            st = work.tile([P, 512], F32, tag="stage", name="st", bufs=4)
            nc.vector.tensor_copy(st[:], ps)
            nc.sync.dma_start(
                ot_d[ec * P:(ec + 1) * P, qc * 512:(qc + 1) * 512], st[:])

        def qproj_chain(qc, pr, qt):
            ps = ppsum.tile([P, 512], F32, tag="proj", name="psq")
            for c in range(DIN):
                nc.tensor.matmul(
                    ps[:],
                    wq_t[:, c, pr * P:(pr + 1) * P],
                    xt_t[:, c, qc * 512:(qc + 1) * 512],
                    start=(c == 0), stop=(c == DIN - 1),
                )
            nc.vector.tensor_scalar_add(qt[:, pr, :], ps[:], bq_t[:, pr:pr + 1])

        # ---- normalize + evict Z^T for one (pr, qc) ----
        # za/zb are staged to SBUF immediately (split across VectorE and
        # ScalarE) so their PSUM banks free up before the next pair's PV
        # needs them; the DMA-bounce broadcast and the normalize muls then
        # run entirely SBUF-side off the PE critical path.
        def normalize(pr, qc, za, zb):
            qsl = slice(qc * 512, (qc + 1) * 512)
            zsa = work.tile([P, 2, 512], F32, tag="zsa", name="zsa", bufs=2)
            nc.vector.tensor_copy(zsa[0:65, 0, :], za[0:65, :])
            nc.scalar.copy(zsa[0:65, 1, :], zb[0:65, :])
            rb = work.tile([P, 2, 512], F32, tag="rbc", name="rb", bufs=2)
            if GPB:
                # shift denom rows to partition 0 (SBUF->SBUF DMA), take
                # reciprocal there, then gpsimd-broadcast to 64 partitions
                rsh = work.tile([P, 2, 512], F32, tag="rsh", name="rsh",
                                bufs=2)
                nc.sync.dma_start(rsh[0:1, :, :], zsa[64:65, :, :])
                rcp = work.tile([P, 2, 512], F32, tag="rcp", name="rcp",
                                bufs=2)
                nc.vector.reciprocal_approx_fast(rcp[0:1, :, :],
                                                 rsh[0:1, :, :])
                nc.gpsimd.partition_broadcast(rb[0:64, :, :],
                                              rcp[0:1, :, :], channels=64)
            else:
                rsc = dramp.tile([2, 512], F32, tag="rsc", name="rsc")
                nc.sync.dma_start(rsc[0:1, :], zsa[64:65, 0, :])
                nc.sync.dma_start(rsc[1:2, :], zsa[64:65, 1, :])
                rbr = work.tile([P, 2, 512], F32, tag="rbc", name="rbr")
                nc.sync.dma_start(rbr[0:64, 0, :],
                                  rsc[0:1, :].to_broadcast((64, 512)))
                nc.sync.dma_start(rbr[0:64, 1, :],
                                  rsc[1:2, :].to_broadcast((64, 512)))
                nc.vector.reciprocal_approx_fast(rb[0:64, :, :],
                                                 rbr[0:64, :, :])
            nc.vector.tensor_mul(zt[pr][0:64, qsl], zsa[0:64, 0, :],
                                 rb[0:64, 0, :])
            zs = work.tile([P, 512], BF16, tag="zstage", name="zs")
            nc.vector.tensor_mul(zs[0:64, :], zsa[0:64, 1, :],
                                 rb[0:64, 1, :])
            nc.sync.dma_start(zt[pr][64:128, qsl], zs[0:64, :])

        # ---- attention steady loop ----
        pv_q = []          # pending PV work: (pq, pr, qc, kc, za, zb)
        zacc = {}          # (pr, qc) -> (za, zb)
        fillers = []       # callables emitting one PE chain each

        def emit_pv(item):
            pq, pr, qc, kc, za, zb = item
            vpr = vt[pr].rearrange("p t (h c) -> p t h c", c=65)
            nc.tensor.matmul(
                za[0:65, :], vpr[:, kc, 0, :], pq[:, 0, :],
                start=(kc == 0), stop=(kc == KC - 1),
            )
            nc.tensor.matmul(
                zb[0:65, :], vpr[:, kc, 1, :], pq[:, 1, :],
                start=(kc == 0), stop=(kc == KC - 1),
            )
            if kc == KC - 1:
                normalize(pr, qc, za, zb)

        FILL_SLOTS = (3, 7, 11)   # kc positions where one filler chain runs

        # first projections: K-proj(pr0, t0) (needs only wk + xt piece 0,
        # both loaded first) runs while wq streams in, then Q-proj(pr0);
        # everything else is JIT inside the stream below
        kproj_chain(0, 0)
        qt_cur = qpool.tile([P, NPAIR, 512], BF16, tag="qt", name="qt0")
        qproj_chain(0, 0, qt_cur)

        for qc in range(QCN):
            qt_use = qt_cur
            # schedule fillers for this qc: Q-proj(qc+1) during pr3,
            # O-proj(qc-1) spread over pr0..pr2
            qnext = [None]
            if qc + 1 < QCN:
                qt_next = qpool.tile([P, NPAIR, 512], BF16, tag="qt",
                                     name=f"qt{(qc + 1) % 2}")
                qnext[0] = qt_next
            for pr in range(NPAIR):
                qsl = slice(qc * 512, (qc + 1) * 512)
                za = zpsum.tile([P, 512], F32, tag="z", name=f"za{pr}_{qc}")
                zb = zpsum.tile([P, 512], F32, tag="z", name=f"zb{pr}_{qc}")
                zacc[(pr, qc)] = (za, zb)
                for kc in range(KC):
                    sq = spsum.tile([P, 2, 512], F32, tag="sc", name="sq")
                    nc.tensor.matmul(
                        sq[:, 0, :],
                        kt[pr][0:64, kc * P:(kc + 1) * P],
                        qt_use[0:64, pr, :],
                        start=True, stop=True,
                    )
                    nc.tensor.matmul(
                        sq[:, 1, :],
                        kt[pr][64:128, kc * P:(kc + 1) * P],
                        qt_use[64:128, pr, :],
                        start=True, stop=True,
                    )
                    pq = probs_pool.tile([P, 2, 512], BF16, tag="pq",
                                         name="pq")
                    nc.scalar.activation(pq[:], sq[:], EXP, scale=0.125)
                    if qc == 0:
                        # just-in-time startup fillers (AFTER the slot's
                        # scores/ACT so DMA-gated chains never block the
                        # scores stream): V-proj chunks feed PV two slots
                        # later; K-proj(t+1) lands before scores(4(t+1));
                        # the next pair's Q-proj/K-proj land near this
                        # pair's end
                        if pr == 0:
                            vproj_chain(kc)
                        if kc in (2, 6, 10):
                            kproj_chain(pr, kc // 4 + 1)
                        if pr < NPAIR - 1 and kc == 12:
                            qproj_chain(0, pr + 1, qt_use)
                        if pr < NPAIR - 1 and kc == 14:
                            kproj_chain(pr + 1, 0)
                    pv_q.append((pq, pr, qc, kc, za, zb))
                    if len(pv_q) > LAG:
                        emit_pv(pv_q.pop(0))
                    if kc in FILL_SLOTS and fillers:
                        fillers.pop(0)()
                # queue fillers now that this pair's scores are done
                if pr == 2 and qnext[0] is not None:
                    qt_next = qnext[0]
                    for fpr in range(NPAIR):
                        fillers.append(
                            lambda q=qc + 1, p=fpr, t=qt_next:
                            qproj_chain(q, p, t))
                if pr == 0 and qc > 0:
                    for ec in range(DIN):
                        fillers.append(
                            lambda q=qc - 1, e=ec: oproj_chain(q, e))
            if qnext[0] is not None:
                qt_cur = qnext[0]

        # drain PV queue, remaining fillers, and final O-proj
        while pv_q:
            emit_pv(pv_q.pop(0))
        while fillers:
            fillers.pop(0)()
        # ---- overlapped final O-proj: six accumulators (2x2 spsum banks
        # + 2 ppsum) run their zt[0..2] partials while the last pair's
        # normalize is still in flight; only the dc=3 matmuls wait on it.
        # Evictions alternate ScalarE/VectorE to drain twice as fast. ----
        qlast = slice((QCN - 1) * 512, QCN * 512)
        s1 = spsum.tile([P, 2, 512], F32, tag="sc", name="tailA")
        s2 = spsum.tile([P, 2, 512], F32, tag="sc", name="tailB")
        p1 = ppsum.tile([P, 512], F32, tag="proj", name="tailC")
        p2 = ppsum.tile([P, 512], F32, tag="proj", name="tailD")
        accs = [s1[:, 0, :], s1[:, 1, :], s2[:, 0, :], s2[:, 1, :],
                p1[:], p2[:]]
        for ec in range(6):
            for dc in range(NPAIR - 1):
                nc.tensor.matmul(
                    accs[ec], wo_t[:, dc, ec * P:(ec + 1) * P],
                    zt[dc][:, qlast], start=(dc == 0), stop=False)
        for ec in range(6):
            nc.tensor.matmul(
                accs[ec], wo_t[:, NPAIR - 1, ec * P:(ec + 1) * P],
                zt[NPAIR - 1][:, qlast], start=False, stop=True)
            st = work.tile([P, 512], F32, tag="stage", name="st", bufs=4)
            if ec % 2 == 0:
                nc.vector.tensor_copy(st[:], accs[ec])
            else:
                nc.scalar.copy(st[:], accs[ec])
            nc.sync.dma_start(ot_d[ec * P:(ec + 1) * P, qlast], st[:])
        for ec in range(6, DIN):
            ps = spsum.tile([P, 2, 512], F32, tag="sc", name="psot")
            for dc in range(NPAIR):
                nc.tensor.matmul(
                    ps[:, 0, :], wo_t[:, dc, ec * P:(ec + 1) * P],
                    zt[dc][:, qlast], start=(dc == 0), stop=(dc == NPAIR - 1))
            st = work.tile([P, 512], F32, tag="stage", name="st", bufs=4)
            if ec % 2 == 0:
                nc.vector.tensor_copy(st[:], ps[:, 0, :])
            else:
                nc.scalar.copy(st[:], ps[:, 0, :])
            nc.sync.dma_start(ot_d[ec * P:(ec + 1) * P, qlast], st[:])

    nc.compile()
    return nc


# ---------------- host-side entry point ----------------

BF = ml_dtypes.bfloat16
_B, _S, _D, _H = 4, 2048, 1024, 16
_DL = _D // 2
_NC_CACHE = None


def _get_nc():
    global _NC_CACHE
    if _NC_CACHE is None:
        _NC_CACHE = build_attention_nc(S=_S, D=_D, DL=_DL)
    return _NC_CACHE


def kernel(X, Wq, bq, Wk, bk, Wv, bv, Wo, bo):
    """Full-input multi-head attention on 8 TRN2 NeuronCores.

    Sharding: core c handles batch c//2 and head-half c%2 (8 heads).
    Each core returns a partial O^T (its heads' contribution); the host
    sums the two partials per batch and adds bo. No collectives.
    """
    from concourse.bass_utils import run_bass_kernel_spmd

    X = np.asarray(X, dtype=np.float32)
    bo = np.asarray(bo, dtype=np.float32)
    wq_f = np.asarray(Wq, dtype=np.float32)
    wk_f = np.asarray(Wk, dtype=np.float32)
    wv_f = np.asarray(Wv, dtype=np.float32)
    wo_f = np.asarray(Wo, dtype=np.float32)
    bq_f = np.asarray(bq, dtype=np.float32)
    bk_f = np.asarray(bk, dtype=np.float32)
    bv_f = np.asarray(bv, dtype=np.float32)

    xts = [np.ascontiguousarray(X[b].T.astype(BF)) for b in range(_B)]
    in_maps = []
    for c in range(8):
        b, hh = c // 2, c % 2
        dsl = slice(hh * _DL, (hh + 1) * _DL)
        in_maps.append({
            "xt": xts[b],
            "wq": np.ascontiguousarray(wq_f[:, dsl].astype(BF)),
            "wk": np.ascontiguousarray(wk_f[:, dsl].astype(BF)),
            "wv": np.ascontiguousarray(wv_f[:, dsl].astype(BF)),
            "wo": np.ascontiguousarray(wo_f[dsl, :].astype(BF)),
            "bq": np.ascontiguousarray(bq_f[dsl]),
            "bk": np.ascontiguousarray(bk_f[dsl]),
            "bv": np.ascontiguousarray(bv_f[dsl]),
        })

    nc = _get_nc()
    res = run_bass_kernel_spmd(nc, in_maps, list(range(8)))

    out = np.empty((_B, _S, _D), np.float32)
    for b in range(_B):
        acc = res.results[2 * b]["ot"] + res.results[2 * b + 1]["ot"]
        out[b] = acc.T + bo[None, :]
    return out
